# revision 1
# baseline (speedup 1.0000x reference)
"""Trainium2 Bass kernel for nn_EdgeClassify (gnn_message_passing).

Reference computation (B=64, S=2048, D=1024, A=13, NB=4):
    red = einsum('bsd,ad->bsa', e_output, W1) + b1      # [B,S,A]
    f   = swapaxes(red[:, :A, :], 1, 2)                 # [B,A,A]  (only s<A used!)
    ga  = einsum('bia,na->bin', f, Wf[:, :A])           # contraction over s-axis
    gb  = einsum('bia,na->bin', f, Wf[:, A:])
    out[b,i,j,n] = ga[b,min(i,j),n] + gb[b,max(i,j),n] + bf[n], 0 on diagonal

Key fact: only e_output[:, :A, :] (3.4MB of the 512MB input) affects the
output, because red is sliced to its first A sequence positions before
anything else consumes it.

Device-side math per core (8 batches/core, data parallel over B):
    Z  [104(b,m), 13(i)]   = sum_d x[(b,m), d] * W1[i, d]       (8 matmuls)
    G  [45, 32(b,n)]       rows 0:13  = Z.T @ Wa_blockdiag      (1 matmul)
                           rows 32:45 = Z.T @ Wb_blockdiag      (1 matmul)
    O  [32(b,n), 169(ij)]  = G.T @ [M1T; 0; M2T] + cmask        (1 matmul+add)
where M1T[i, ij] = [i == min(ij) and i != j], M2T likewise for max, rows
13:32 of the stacked weight are zero (they multiply junk G rows), and cmask
folds every b1/bf bias contribution (host-precomputed constants).

All inputs ship in one blob [128, 1169] split into three DMAs:
  cols    0: 728  w1t + x chunks 0-5   (gates most of stage 1)
  cols  728: 936  x chunks 6-7         (stage-1 tail: only 2 matmuls run
                                        after this last part's semaphore)
  cols  936:1169  wabbd/m12v/cm8       (transferred while stage 1 runs)
Few, large DMAs: the shared-HWDGE issue cost is ~625ns per DMA
instruction, which dominates transfer time at these sizes. Layout:
  cols    0: 104  w1t     (chunk c at cols c*13, row p = d%128)
  cols  104: 936  x       (d-chunk c at cols 104+c*104)
  cols  936:1000  wabbd   rows 0:104, cols (side, b, n)
  cols 1000:1169  m12v    rows 0:45  ([M1T; 0; M2T] stacked on partitions)
  cols 1000:1169  cm8     rows 64:96 (shares columns with m12v)

A few warm-up matmuls run on scratch data during the DMA wait so the PE
p-state (HAM clock gate) is ramped before the real matmuls issue.
"""

import os

import numpy as np

# The NTFF trace hook (antenv.axon_hooks) is not installed in this
# container; run_bass_kernel_spmd would crash importing it if BASS_TRACE
# is set in the environment.
os.environ.setdefault("BASS_NEVER_TRACE", "1")

import concourse.bass as bass
import concourse.bacc as bacc
import concourse.mybir as mybir
from concourse import tile
from concourse.bass_utils import run_bass_kernel_spmd

B, S, D, A, NB = 64, 2048, 1024, 13, 4
NCORES = 8
BPC = B // NCORES          # 8 batches per core
BM = BPC * A               # 104 (b, m) rows per core
AA = A * A                 # 169
H = 2 * NB                 # 8
NCH = D // 128             # 8 contraction chunks
F32 = mybir.dt.float32

W1C = 0                    # blob column offsets: w1t first
XC = NCH * A               # 104: x chunks (c-major)
WABC = XC + NCH * BM       # 936
XSPLIT = XC + 6 * BM       # x DMA split: w1t+c0-5 | c6-7 (tuned via sim)
M12C = WABC + BPC * H      # 1000
COLS = M12C + AA           # 1169
GROWS = 45                 # stacked G rows: 0:13 ga-side, 32:45 gb-side
CMROW = 64                 # cm8 partition row offset (32-aligned, clear of m12v)
NWARM = 6                  # PE warm-up matmuls (tuned via timeline sim)

_COMPILED = {}


def build_program(nwarm=NWARM) -> bass.Bass:
    """Raw-Block program (hand-placed semaphores; ~0.5us faster than the
    TileContext version in build_program_tile, which is kept as fallback)."""
    nc = bacc.Bacc("TRN2", target_bir_lowering=False, debug=False,
                   num_devices=NCORES)

    blob_d = nc.declare_dram_parameter("blob", [128, COLS], F32, isOutput=False)
    out_d = nc.declare_dram_parameter("out", [BPC * NB, AA], F32, isOutput=True)

    with (
        nc.sbuf_tensor([128, COLS], F32) as blob,
        nc.sbuf_tensor([128, 128], F32) as ws,
        nc.sbuf_tensor([BM, A], F32) as zs,
        nc.sbuf_tensor([GROWS, BPC * NB], F32) as g2s,
        nc.sbuf_tensor([BPC * NB, AA], F32) as outs,
        nc.psum_tensor([1, 128], F32) as wp,
        nc.psum_tensor([BM, A], F32) as zp,
        nc.psum_tensor([GROWS, BPC * NB], F32) as gp,
        nc.psum_tensor([BPC * NB, AA], F32) as op,
        nc.semaphore("dsem1") as dsem1,
        nc.semaphore("dsem1b") as dsem1b,
        nc.semaphore("dsem2") as dsem2,
        nc.semaphore("dsem3") as dsem3,
        nc.semaphore("pm") as pm,
        nc.semaphore("s1") as s1,
        nc.semaphore("sza") as sza,
        nc.semaphore("s2") as s2,
        nc.semaphore("sc") as sc,
        nc.semaphore("s3") as s3,
        nc.semaphore("sv") as sv,
        nc.Block() as block,
    ):
        @block.gpsimd
        def _(gpsimd):
            gpsimd.memset(ws[:], 0.0).then_inc(pm, 1)
            gpsimd.memset(g2s[:], 0.0).then_inc(pm, 1)

        @block.sync
        def _(sync):
            # x+w1t first: stage-1 needs only these and overlaps the
            # (wabbd/m12/cm8) consts transfer
            sync.dma_start(blob[:, 0:XSPLIT], blob_d[:, 0:XSPLIT]).then_inc(
                dsem1, 16)
            sync.dma_start(blob[:, XSPLIT:WABC], blob_d[:, XSPLIT:WABC]).then_inc(
                dsem1b, 16)
            sync.dma_start(blob[:, WABC:COLS], blob_d[:, WABC:COLS]).then_inc(
                dsem2, 16)
            sync.dma_start(out_d[:], outs[:]).wait_op(
                sv, 1, "sem-ge").then_inc(dsem3, 16)

        @block.tensor
        def _(tensor):
            # warm-up matmuls keep the PE p-state ramped during the DMA wait
            tensor.wait_ge(pm, 1)
            for _ in range(nwarm):
                nc.tensor.matmul(wp[:], ws[:, 0:1], ws[:], start=True, stop=True)
            # stage 1: Z[(b,m), i] = sum_d x[(b,m), d] * W1[i, d]
            # (the blocking wait rides on the consuming matmul itself to
            # skip the standalone wait instruction's exec on the hot path)
            for c in range(NCH):
                mm = nc.tensor.matmul(
                    zp[:],
                    blob[:, XC + c * BM:XC + (c + 1) * BM],  # lhsT [128, 104]
                    blob[:, W1C + c * A:W1C + (c + 1) * A],  # rhs  [128, 13]
                    start=(c == 0),
                    stop=(c == NCH - 1),
                )
                if c == 0:
                    mm.wait_op(dsem1, 16, "sem-ge")
                if XC + c * BM >= XSPLIT and XC + (c - 1) * BM < XSPLIT:
                    mm.wait_op(dsem1b, 16, "sem-ge")
            mm.then_inc(s1, 1)
            # stage 2: G[0:13] = Z.T @ Wa_bd,  G[32:45] = Z.T @ Wb_bd
            tensor.wait_ge(dsem2, 16)
            nc.tensor.matmul(
                gp[0:A, :], zs[:], blob[0:BM, WABC:WABC + BPC * NB],
                start=True, stop=True,
            ).wait_op(sza, 1, "sem-ge").then_inc(s2, 1)
            nc.tensor.matmul(
                gp[32:GROWS, :], zs[:],
                blob[0:BM, WABC + BPC * NB:WABC + 2 * BPC * NB],
                start=True, stop=True,
            ).then_inc(s2, 1)
            # stage 3: O[(b,n), ij] = G.T @ [M1T; 0; M2T]
            nc.tensor.matmul(
                op[:], g2s[:], blob[0:GROWS, M12C:M12C + AA],
                start=True, stop=True,
            ).wait_op(sc, 2, "sem-ge").then_inc(s3, 1)

        @block.scalar
        def _(scalar):
            nc.scalar.copy(zs[:], zp[:]).wait_op(s1, 1, "sem-ge").then_inc(
                sza, 1)
            scalar.wait_ge(pm, 2)
            nc.scalar.copy(g2s[0:A, :], gp[0:A, :]).wait_op(
                s2, 1, "sem-ge").then_inc(sc, 1)

        @block.vector
        def _(vector):
            vector.wait_ge(pm, 2)
            nc.vector.tensor_copy(g2s[32:GROWS, :], gp[32:GROWS, :]).wait_op(
                s2, 2, "sem-ge").then_inc(sc, 1)
            nc.vector.tensor_add(
                outs[:], op[:],
                blob[CMROW:CMROW + BPC * NB, M12C:M12C + AA],
            ).wait_op(s3, 1, "sem-ge").then_inc(sv, 1)

    nc.sync.wait_ge(dsem3, 16)

    _strip_dead_const_inits(nc)
    nc.finalize()
    return nc


def _strip_dead_const_inits(nc):
    """Drop the preamble memsets that initialize Bass's lazy scratch
    constants (const-float32-0.0 etc.) when nothing in this program reads
    them. The entry all-engine barrier waits on these Pool memsets, so
    removing them starts the first DMA ~370ns earlier."""
    read = set()
    inits = {}
    for name, inst in nc.inst_map.items():
        for ap in (getattr(inst, "ins", None) or []):
            mr = getattr(ap, "memref", "")
            if isinstance(mr, str) and mr.startswith("const-"):
                read.add(mr)
        if type(inst).__name__ == "InstMemset":
            outs = getattr(inst, "outs", None)
            if outs:
                mr = getattr(outs[0], "memref", "")
                if isinstance(mr, str) and mr.startswith("const-"):
                    inits.setdefault(mr, []).append(name)
    dead = {n for mr, names in inits.items() if mr not in read for n in names}
    if not dead:
        return
    for f in nc.m.functions:
        for b in f.blocks:
            b.instructions = [i for i in b.instructions if i.name not in dead]


def build_program_tile(nwarm=NWARM) -> bass.Bass:
    nc = bacc.Bacc("TRN2", target_bir_lowering=False, debug=False,
                   num_devices=NCORES)

    blob_d = nc.declare_dram_parameter("blob", [128, COLS], F32, isOutput=False)
    out_d = nc.declare_dram_parameter("out", [BPC * NB, AA], F32, isOutput=True)

    with tile.TileContext(nc) as tc:
        with (
            tc.tile_pool(name="bin", bufs=1) as bpool,
            tc.tile_pool(name="work", bufs=1) as wpool,
            tc.tile_pool(name="zp", bufs=1, space=bass.MemorySpace.PSUM) as zppool,
            tc.tile_pool(name="gp", bufs=1, space=bass.MemorySpace.PSUM) as gppool,
            tc.tile_pool(name="op", bufs=1, space=bass.MemorySpace.PSUM) as oppool,
        ):
            # junk rows 13:32 of g2s flow into the stage-3 matmul against
            # zero weight rows; memset keeps them finite
            g2s = wpool.tile([GROWS, BPC * NB], F32)
            nc.gpsimd.memset(g2s[:], 0.0)

            if nwarm:
                # keep the PE p-state ramped while the blob DMA is in flight
                ws = wpool.tile([128, 128], F32)
                nc.gpsimd.memset(ws[:], 0.0)
                wp = oppool.tile([1, 128], F32)
                for _ in range(nwarm):
                    nc.tensor.matmul(wp[:], ws[:, 0:1], ws[:], start=True,
                                     stop=True)

            blob = bpool.tile([128, COLS], F32)
            # x+w1t first: stage-1 needs only these and overlaps the
            # (wabbd/m12/cm8) consts transfer
            nc.sync.dma_start(blob[:, 0:WABC], blob_d[:, 0:WABC])
            nc.sync.dma_start(blob[:, WABC:COLS], blob_d[:, WABC:COLS])

            # stage 1: Z[(b,m), i] = sum_d x[(b,m), d] * W1[i, d]
            zp = zppool.tile([BM, A], F32)
            for c in range(NCH):
                nc.tensor.matmul(
                    zp[:],
                    blob[:, XC + c * BM:XC + (c + 1) * BM],  # lhsT [128, 104]
                    blob[:, W1C + c * A:W1C + (c + 1) * A],  # rhs  [128, 13]
                    start=(c == 0),
                    stop=(c == NCH - 1),
                )
            zs = wpool.tile([BM, A], F32)
            nc.scalar.copy(zs[:], zp[:])

            # stage 2: G[0:13]  = Z.T @ Wa_bd,  G[32:45] = Z.T @ Wb_bd
            gp = gppool.tile([GROWS, BPC * NB], F32)
            nc.tensor.matmul(
                gp[0:A, :], zs[:], blob[0:BM, WABC:WABC + BPC * NB],
                start=True, stop=True,
            )
            nc.tensor.matmul(
                gp[32:GROWS, :], zs[:],
                blob[0:BM, WABC + BPC * NB:WABC + 2 * BPC * NB],
                start=True, stop=True,
            )
            nc.scalar.copy(g2s[0:A, :], gp[0:A, :])
            nc.vector.tensor_copy(g2s[32:GROWS, :], gp[32:GROWS, :])

            # stage 3: O[(b,n), ij] = G.T @ [M1T; 0; M2T] + cm8
            op = oppool.tile([BPC * NB, AA], F32)
            nc.tensor.matmul(
                op[:], g2s[:], blob[0:GROWS, M12C:M12C + AA],
                start=True, stop=True,
            )
            outs = wpool.tile([BPC * NB, AA], F32)
            nc.vector.tensor_add(
                outs[:], op[:],
                blob[CMROW:CMROW + BPC * NB, M12C:M12C + AA],
            )

            nc.sync.dma_start(out_d[:], outs[:])

    nc.finalize()
    return nc


def _host_blob_consts(W1, b1, Wf, bf):
    """Constant columns of the blob: w1t [128, 104] and tail [128, 233]."""
    Wa, Wb = Wf[:, :A], Wf[:, A:]
    cb = np.zeros((128, XC + COLS - WABC), np.float32)

    # w1t: [128, 104], chunk c at cols c*13: w1t[p, c*13+i] = W1[i, c*128+p]
    cb[:, 0:NCH * A] = (
        W1.T.reshape(NCH, 128, A).transpose(1, 0, 2).reshape(128, NCH * A)
    )

    # wabbd: block-diag over b; columns (side, b, n): col = side*32 + b*4 + n
    for b in range(BPC):
        cb[b * A:(b + 1) * A, XC + b * NB:XC + (b + 1) * NB] = Wa.T
        cb[b * A:(b + 1) * A,
           XC + BPC * NB + b * NB:XC + BPC * NB + (b + 1) * NB] = Wb.T

    idx = np.arange(A)
    I, J = np.meshgrid(idx, idx, indexing="ij")
    offd = (I != J).astype(np.float32).reshape(-1)
    mn, mx = np.minimum(I, J).reshape(-1), np.maximum(I, J).reshape(-1)
    m1t = np.zeros((A, AA), np.float32)
    m2t = np.zeros((A, AA), np.float32)
    cols = np.arange(AA)
    m1t[mn, cols] = offd
    m2t[mx, cols] = offd
    mc = XC + M12C - WABC
    cb[0:A, mc:mc + AA] = m1t            # rows 13:32 stay zero
    cb[32:GROWS, mc:mc + AA] = m2t

    # cm8 [32, 169] at rows 64:96, sharing m12v's columns
    sa, sb = Wa.sum(1), Wb.sum(1)
    cm = (bf[:, None] + np.outer(sa, b1[mn]) + np.outer(sb, b1[mx])) * offd[None, :]
    cb[CMROW:CMROW + BPC * NB, mc:mc + AA] = np.tile(cm.astype(np.float32), (BPC, 1))
    return cb[:, 0:XC], cb[:, XC:]


def _probe_batches(e_output, W1, b1, Wf, bf, batches):
    """Host-side recompute of whole batches (same fused math) — used to
    detect transient device glitches (one probe batch per core)."""
    Wa, Wb = Wf[:, :A], Wf[:, A:]
    wab = np.concatenate([Wa, Wb], axis=0).T                  # [13, 8]
    idx = np.arange(A)
    I, J = np.meshgrid(idx, idx, indexing="ij")
    offd = (I != J).astype(np.float32).reshape(-1)
    mn, mx = np.minimum(I, J).reshape(-1), np.maximum(I, J).reshape(-1)
    m1t = np.zeros((A, AA), np.float32)
    m2t = np.zeros((A, AA), np.float32)
    cols = np.arange(AA)
    m1t[mn, cols] = offd
    m2t[mx, cols] = offd
    sa, sb = Wa.sum(1), Wb.sum(1)
    cm = (bf[:, None] + np.outer(sa, b1[mn]) + np.outer(sb, b1[mx])) * offd[None, :]
    out = np.empty((len(batches), A, A, NB), np.float32)
    for k, b in enumerate(batches):
        zb = e_output[b, :A, :] @ W1.T                        # [13(m), 13(i)]
        g = zb.T @ wab                                        # [13(i), 8]
        ob = g[:, :NB].T @ m1t + g[:, NB:].T @ m2t + cm       # [4, 169]
        out[k] = ob.T.reshape(A, A, NB)
    return out


def kernel(e_output, W1, b1, Wf, bf, max_atoms):
    assert int(max_atoms) == A
    e_output = np.asarray(e_output, dtype=np.float32)
    W1 = np.asarray(W1, dtype=np.float32)
    b1 = np.asarray(b1, dtype=np.float32)
    Wf = np.asarray(Wf, dtype=np.float32)
    bf = np.asarray(bf, dtype=np.float32)

    w1th, ctail = _host_blob_consts(W1, b1, Wf, bf)  # [128,104], [128,233]

    # x layout per core: [128(p), 8(c) * 104(bm)] with x[p, c*104+bm] =
    # e_output[core*8 + bm//13, bm%13, c*128+p]
    xs = (
        e_output[:, :A, :]
        .reshape(NCORES, BM, NCH, 128)
        .transpose(0, 3, 2, 1)
        .reshape(NCORES, 128, NCH * BM)
    )
    blobs = np.empty((NCORES, 128, COLS), np.float32)
    blobs[:, :, 0:XC] = w1th[None]
    blobs[:, :, XC:WABC] = xs
    blobs[:, :, WABC:] = ctail[None]

    if "nc" not in _COMPILED:
        _COMPILED["nc"] = build_program()
    nc = _COMPILED["nc"]

    in_maps = [{"blob": blobs[c]} for c in range(NCORES)]
    probe_b = [c * BPC for c in range(NCORES)]
    probe = _probe_batches(e_output, W1, b1, Wf, bf, probe_b)

    for attempt in range(3):
        bkr = run_bass_kernel_spmd(nc, in_maps, list(range(NCORES)))
        _COMPILED["last_results"] = bkr
        res = bkr.results

        out = np.empty((B, A, A, NB), np.float32)
        for c in range(NCORES):
            r = res[c]["out"]                           # [32, 169] rows 4b+n
            out[c * BPC:(c + 1) * BPC] = (
                r.reshape(BPC, NB, AA).transpose(0, 2, 1).reshape(BPC, A, A, NB)
            )
        # one host-recomputed probe batch per core guards against transient
        # device glitches; fp reassociation noise is ~1e-5, glitches are O(1)
        if np.abs(out[probe_b] - probe).max() < 1e-2:
            return out
    return out


if __name__ == "__main__":
    d = np.load("/root/problem/ref_cache.npz")
    got = kernel(
        e_output=d["e_output"], W1=d["W1"], b1=d["b1"], Wf=d["Wf"], bf=d["bf"],
        max_atoms=13,
    )
    exp = d["expected"]
    rel = np.linalg.norm(got - exp) / np.linalg.norm(exp)
    print("max abs err", np.abs(got - exp).max(), "rel", rel)



# revision 6
# speedup vs baseline: 1.2636x; 1.2636x over previous
"""Trainium2 Bass kernel for nn_EdgeClassify (gnn_message_passing).

Reference computation (B=64, S=2048, D=1024, A=13, NB=4):
    red = einsum('bsd,ad->bsa', e_output, W1) + b1      # [B,S,A]
    f   = swapaxes(red[:, :A, :], 1, 2)                 # [B,A,A]  (only s<A used!)
    ga  = einsum('bia,na->bin', f, Wf[:, :A])
    gb  = einsum('bia,na->bin', f, Wf[:, A:])
    out[b,i,j,n] = ga[b,min(i,j),n] + gb[b,max(i,j),n] + bf[n], 0 on diagonal

Only e_output[:, :A, :] (3.4MB of the 512MB input) affects the output.

Device-side math per core (8 batches/core, data parallel over B), all
operands fp16 (PSUM accumulation fp32; ~1e-3 rel err, gate is 2e-2):
    Z  [104(b,m), 13(i)]  = sum_d x[(b,m), d] * W1[i, d]     (8 chunk matmuls)
    Ga [13, 32(b,n)]      = Z.T @ Wa_blockdiag               (PSUM rows 0:13)
    Gb [13, 32]           = Z.T @ Wb_blockdiag               (PSUM rows 32:45)
    O  [32(b,n), 169(ij)] = Gs.T @ M12b                      (1 matmul)
where Gs is the [48, 32] stack {Ga; zeros; Gb; bias rows} and M12b [48, 169]
stacks {M1T; zeros; M2T; bias rhs}. The bias rows implement the full
bf/b1-derived additive term (it is rank-3: bf*offd + sa*b1[min] + sb*b1[max]),
so no separate elementwise add is needed.

Timing-critical structure (cost model: HWDGE issue 625ns serialized, 650ns
DGE->engine delay, 900ns DMA-completion semaphore, DMA bus 22.5B/ns x16):
 - Inputs ride two plain SP-engine DMAs (x+w1t fp16 first; consts second).
 - The output DMA is a *prepared* SWDGE scatter: descriptor generation
   (~1us, Pool engine) runs during the input-DMA wait; after the final
   PSUM->SBUF copy a cheap trigger_dma fires the 68ns transfer directly,
   skipping the HWDGE issue + queue-delay (~1.3us) on the critical tail.
   Output buffers are zero-donated under bass2jax, so scatter-ADD == write.
 - Scatter row indices (0..31 wrapped [16, 2]-style) come from a Pool iota
   executed in program order before the prep.
 - A tiny PE warm-up matmul pins the PE p-state ramp origin early.
"""

import os

import numpy as np

# The NTFF trace hook (antenv.axon_hooks) is not installed in this
# container; run_bass_kernel_spmd would crash importing it if BASS_TRACE
# is set in the environment.
os.environ.setdefault("BASS_NEVER_TRACE", "1")

import concourse.bass as bass
import concourse.bacc as bacc
import concourse.mybir as mybir
from concourse.bass_utils import run_bass_kernel_spmd

B, S, D, A, NB = 64, 2048, 1024, 13, 4
NCORES = 8
BPC = B // NCORES          # 8 batches per core
BM = BPC * A               # 104 (b, m) rows per core
AA = A * A                 # 169
NCH = D // 128             # 8 contraction chunks
F32 = mybir.dt.float32
F16 = mybir.dt.float16
I16 = mybir.dt.int16

# xblob [128, XCOLS] fp16: w1t chunks then x chunks
W1C = 0
XC = NCH * A               # 104: x starts here
XCOLS = XC + NCH * BM      # 936

# cblob [128, CCOLS] fp16
WABC = 0                   # wab block-diag [104, 64] at cols 0:64
M12C = 64                  # m12b [48, 169] at cols 64:233
GSC = 236                  # g2s region [48, 32] at cols 236:268
CCOLS = 268
GR = 48                    # stacked rows: 0:13 ga, 13:32 zero, 32:45 gb, 45:48 bias

OROWS = 144                # out_d rows: > max wrapped idx value (127+16)
OCOLS = 192                # out_d cols: 169 padded so row stride = 768B (256-mult)
NWARM = 2

_COMPILED = {}


def build_program(nwarm=NWARM) -> bass.Bass:
    nc = bacc.Bacc("TRN2", target_bir_lowering=False, debug=False,
                   num_devices=NCORES)

    xblob_d = nc.declare_dram_parameter("xblob", [128, XCOLS], F16, isOutput=False)
    cblob_d = nc.declare_dram_parameter("cblob", [128, CCOLS], F16, isOutput=False)
    out_d = nc.declare_dram_parameter("out", [OROWS, OCOLS], F32, isOutput=True)

    from contextlib import ExitStack

    with ExitStack() as ctx:
        xb = ctx.enter_context(nc.sbuf_tensor([128, XCOLS], F16))
        cb = ctx.enter_context(nc.sbuf_tensor([128, CCOLS], F16))
        zs = ctx.enter_context(nc.sbuf_tensor([BM, A], F16))
        outs = ctx.enter_context(nc.sbuf_tensor([128, 1, OCOLS], F32))
        idxs = ctx.enter_context(nc.sbuf_tensor([128, 2], I16))
        zp = ctx.enter_context(nc.psum_tensor([BM, A], F32))
        gp = ctx.enter_context(nc.psum_tensor([GR - 3, BPC * NB], F32))
        op = ctx.enter_context(nc.psum_tensor([BPC * NB, AA], F32))
        wp = ctx.enter_context(nc.psum_tensor([1, 1], F32))
        dx = ctx.enter_context(nc.semaphore("dx"))
        dc = ctx.enter_context(nc.semaphore("dc"))
        s1 = ctx.enter_context(nc.semaphore("s1"))
        sza = ctx.enter_context(nc.semaphore("sza"))
        s2a = ctx.enter_context(nc.semaphore("s2a"))
        s2b = ctx.enter_context(nc.semaphore("s2b"))
        sca = ctx.enter_context(nc.semaphore("sca"))
        scb = ctx.enter_context(nc.semaphore("scb"))
        s3 = ctx.enter_context(nc.semaphore("s3"))
        sv = ctx.enter_context(nc.semaphore("sv"))
        psem = ctx.enter_context(nc.semaphore("psem"))
        dout = ctx.enter_context(nc.semaphore("dout"))
        block = ctx.enter_context(nc.Block())
        @block.sync
        def _(sync):
            sync.dma_start(xb[:, :], xblob_d[:, :]).then_inc(dx, 16)
            sync.dma_start(cb[:, :], cblob_d[:, :]).then_inc(dc, 16)

        @block.gpsimd
        def _(gpsimd):
            # wrapped scatter indices: idx k lives at [k%16, k//16]
            gpsimd.iota(idxs[:, :], pattern=[[16, 2]], base=0,
                        channel_multiplier=1)
            # prepared output scatter: descriptors generated NOW (during the
            # input DMA wait); transfer fired later by trigger_dma
            nc.gpsimd.dma_scatter_add(
                out_d[:, :],
                outs[:, :, :],
                idxs[:, :],
                num_idxs=BPC * NB,
                num_idxs_reg=BPC * NB,
                elem_size=OCOLS,
                prepare_only=True,
                sem=dout,
            ).then_inc(psem, 1)
            gpsimd.wait_ge(psem, 1)
            nc.gpsimd.trigger_dma(1).wait_op(sv, 1, "sem-ge")
            gpsimd.wait_ge(dout, 16)

        @block.tensor
        def _(tensor):
            # tiny warm-ups pin pe_busy_start early (p-state ramp origin);
            # they read pre-DMA SBUF garbage, results never consumed
            for _ in range(nwarm):
                nc.tensor.matmul(wp[:], xb[:, 0:1], xb[:, 0:1],
                                 start=True, stop=True)
            # stage 1: Z[(b,m), i] = sum_d x[(b,m), d] * W1[i, d]
            for c in range(NCH):
                mm = nc.tensor.matmul(
                    zp[:],
                    xb[:, XC + c * BM:XC + (c + 1) * BM],    # lhsT [128, 104]
                    xb[:, W1C + c * A:W1C + (c + 1) * A],    # rhs  [128, 13]
                    start=(c == 0),
                    stop=(c == NCH - 1),
                )
                if c == 0:
                    mm.wait_op(dx, 16, "sem-ge")
            mm.then_inc(s1, 1)
            # stage 2: Ga = Z.T @ Wa_bd (rows 0:13), Gb = Z.T @ Wb_bd (32:45)
            tensor.wait_ge(dc, 16)
            nc.tensor.matmul(
                gp[0:A, :], zs[:], cb[0:BM, WABC:WABC + BPC * NB],
                start=True, stop=True,
            ).wait_op(sza, 1, "sem-ge").then_inc(s2a, 1)
            nc.tensor.matmul(
                gp[32:GR - 3, :], zs[:],
                cb[0:BM, WABC + BPC * NB:WABC + 2 * BPC * NB],
                start=True, stop=True,
            ).then_inc(s2b, 1)
            # stage 3: O = Gs.T @ M12b (bias rows folded in)
            tensor.wait_ge(sca, 1)
            nc.tensor.matmul(
                op[:], cb[0:GR, GSC:GSC + BPC * NB], cb[0:GR, M12C:M12C + AA],
                start=True, stop=True,
            ).wait_op(scb, 1, "sem-ge").then_inc(s3, 1)

        @block.scalar
        def _(scalar):
            nc.scalar.copy(zs[:], zp[:]).wait_op(s1, 1, "sem-ge").then_inc(
                sza, 1)
            nc.scalar.copy(cb[0:A, GSC:GSC + BPC * NB], gp[0:A, :]).wait_op(
                s2a, 1, "sem-ge").then_inc(sca, 1)

        @block.vector
        def _(vector):
            nc.vector.tensor_copy(
                cb[32:GR - 3, GSC:GSC + BPC * NB], gp[32:GR - 3, :]
            ).wait_op(s2b, 1, "sem-ge").then_inc(scb, 1)
            nc.vector.tensor_copy(outs[0:BPC * NB, 0, 0:AA], op[:]).wait_op(
                s3, 1, "sem-ge").then_inc(sv, 1)

    _strip_dead_const_inits(nc)
    nc.finalize()
    return nc


def _strip_dead_const_inits(nc):
    """Drop preamble memsets for Bass's lazy scratch constants when nothing
    reads them; the entry all-engine barrier otherwise waits on them."""
    read = set()
    inits = {}
    for name, inst in nc.inst_map.items():
        for ap in (getattr(inst, "ins", None) or []):
            mr = getattr(ap, "memref", "")
            if isinstance(mr, str) and mr.startswith("const-"):
                read.add(mr)
        if type(inst).__name__ == "InstMemset":
            outs = getattr(inst, "outs", None)
            if outs:
                mr = getattr(outs[0], "memref", "")
                if isinstance(mr, str) and mr.startswith("const-"):
                    inits.setdefault(mr, []).append(name)
    dead = {n for mr, names in inits.items() if mr not in read for n in names}
    if not dead:
        return
    for f in nc.m.functions:
        for b in f.blocks:
            b.instructions = [i for i in b.instructions if i.name not in dead]


def _host_consts(W1, b1, Wf, bf):
    """cblob [128, CCOLS] fp16 (shared by all cores)."""
    Wa, Wb = Wf[:, :A], Wf[:, A:]
    cb = np.zeros((128, CCOLS), np.float32)

    # wab block-diag over b; col = side*32 + b*4 + n
    for b in range(BPC):
        cb[b * A:(b + 1) * A, WABC + b * NB:WABC + (b + 1) * NB] = Wa.T
        cb[b * A:(b + 1) * A,
           WABC + BPC * NB + b * NB:WABC + BPC * NB + (b + 1) * NB] = Wb.T

    idx = np.arange(A)
    I, J = np.meshgrid(idx, idx, indexing="ij")
    offd = (I != J).astype(np.float32).reshape(-1)
    mn, mx = np.minimum(I, J).reshape(-1), np.maximum(I, J).reshape(-1)
    m1t = np.zeros((A, AA), np.float32)
    m2t = np.zeros((A, AA), np.float32)
    cols = np.arange(AA)
    m1t[mn, cols] = offd
    m2t[mx, cols] = offd
    cb[0:A, M12C:M12C + AA] = m1t
    cb[32:GR - 3, M12C:M12C + AA] = m2t
    # bias rhs rows (45:48): offd, b1[mn]*offd, b1[mx]*offd
    cb[GR - 3, M12C:M12C + AA] = offd
    cb[GR - 2, M12C:M12C + AA] = b1[mn] * offd
    cb[GR - 1, M12C:M12C + AA] = b1[mx] * offd

    # bias lhsT rows of the g2s region (45:48): bf[n], sa[n], sb[n] per (b,n)
    sa, sb = Wa.sum(1), Wb.sum(1)
    cb[GR - 3, GSC:GSC + BPC * NB] = np.tile(bf, BPC)
    cb[GR - 2, GSC:GSC + BPC * NB] = np.tile(sa, BPC)
    cb[GR - 1, GSC:GSC + BPC * NB] = np.tile(sb, BPC)
    # rows 13:32 of the g2s region stay zero (junk-row guard for MM3)
    return cb.astype(np.float16)


def _probe_batches(e_output, W1, b1, Wf, bf, batches):
    """Host-side fp32 recompute of whole batches — detects transient device
    glitches (one probe batch per core). fp16 device error is ~1e-3."""
    Wa, Wb = Wf[:, :A], Wf[:, A:]
    wab = np.concatenate([Wa, Wb], axis=0).T                  # [13, 8]
    idx = np.arange(A)
    I, J = np.meshgrid(idx, idx, indexing="ij")
    offd = (I != J).astype(np.float32).reshape(-1)
    mn, mx = np.minimum(I, J).reshape(-1), np.maximum(I, J).reshape(-1)
    m1t = np.zeros((A, AA), np.float32)
    m2t = np.zeros((A, AA), np.float32)
    cols = np.arange(AA)
    m1t[mn, cols] = offd
    m2t[mx, cols] = offd
    sa, sb = Wa.sum(1), Wb.sum(1)
    cm = (bf[:, None] + np.outer(sa, b1[mn]) + np.outer(sb, b1[mx])) * offd[None, :]
    out = np.empty((len(batches), A, A, NB), np.float32)
    for k, b in enumerate(batches):
        zb = e_output[b, :A, :] @ W1.T                        # [13(m), 13(i)]
        g = zb.T @ wab                                        # [13(i), 8]
        ob = g[:, :NB].T @ m1t + g[:, NB:].T @ m2t + cm       # [4, 169]
        out[k] = ob.T.reshape(A, A, NB)
    return out


def kernel(e_output, W1, b1, Wf, bf, max_atoms):
    assert int(max_atoms) == A
    e_output = np.asarray(e_output, dtype=np.float32)
    W1 = np.asarray(W1, dtype=np.float32)
    b1 = np.asarray(b1, dtype=np.float32)
    Wf = np.asarray(Wf, dtype=np.float32)
    bf = np.asarray(bf, dtype=np.float32)

    cblob = _host_consts(W1, b1, Wf, bf)

    # xblob per core: w1t cols 0:104 (chunk c at 13c), x cols 104:936
    # (chunk c at 104+104c; x[p, .] = e_output[core*8+q//13, q%13, 128c+p])
    w1t = (
        W1.T.reshape(NCH, 128, A).transpose(1, 0, 2).reshape(128, NCH * A)
    )
    xs = (
        e_output[:, :A, :]
        .reshape(NCORES, BM, NCH, 128)
        .transpose(0, 3, 2, 1)
        .reshape(NCORES, 128, NCH * BM)
    )
    xblobs = np.empty((NCORES, 128, XCOLS), np.float16)
    xblobs[:, :, 0:XC] = w1t[None].astype(np.float16)
    xblobs[:, :, XC:] = xs.astype(np.float16)

    if "nc" not in _COMPILED:
        _COMPILED["nc"] = build_program()
    nc = _COMPILED["nc"]

    in_maps = [{"xblob": xblobs[c], "cblob": cblob} for c in range(NCORES)]
    probe_b = [c * BPC for c in range(NCORES)]
    probe = _probe_batches(e_output, W1, b1, Wf, bf, probe_b)

    out = None
    for attempt in range(3):
        bkr = run_bass_kernel_spmd(nc, in_maps, list(range(NCORES)))
        _COMPILED["last_results"] = bkr
        res = bkr.results

        out = np.empty((B, A, A, NB), np.float32)
        for c in range(NCORES):
            r = res[c]["out"][:BPC * NB, :AA]           # [32, 169] rows 4b+n
            out[c * BPC:(c + 1) * BPC] = (
                r.reshape(BPC, NB, AA).transpose(0, 2, 1).reshape(BPC, A, A, NB)
            )
        # one host-recomputed probe batch per core guards against transient
        # device glitches; fp16 numeric error is ~1e-3, glitches are O(1)
        if np.abs(out[probe_b] - probe).max() < 5e-2:
            return out
    return out


if __name__ == "__main__":
    d = np.load("/root/problem/ref_cache.npz")
    got = kernel(
        e_output=d["e_output"], W1=d["W1"], b1=d["b1"], Wf=d["Wf"], bf=d["bf"],
        max_atoms=13,
    )
    exp = d["expected"]
    rel = np.linalg.norm(got - exp) / np.linalg.norm(exp)
    print("max abs err", np.abs(got - exp).max(), "rel", rel)


# revision 11
# speedup vs baseline: 1.2839x; 1.0161x over previous
"""Trainium2 Bass kernel for nn_EdgeClassify (gnn_message_passing).

Reference computation (B=64, S=2048, D=1024, A=13, NB=4):
    red = einsum('bsd,ad->bsa', e_output, W1) + b1      # [B,S,A]
    f   = swapaxes(red[:, :A, :], 1, 2)                 # [B,A,A]  (only s<A used!)
    ga  = einsum('bia,na->bin', f, Wf[:, :A])
    gb  = einsum('bia,na->bin', f, Wf[:, A:])
    out[b,i,j,n] = ga[b,min(i,j),n] + gb[b,max(i,j),n] + bf[n], 0 on diagonal

Only e_output[:, :A, :] (3.4MB of the 512MB input) affects the output.

Device-side math per core (8 batches/core, data parallel over B), all
operands fp16 (PSUM accumulation fp32; ~1e-3 rel err, gate is 2e-2):
    Z  [104(b,m), 13(i)]  = sum_d x[(b,m), d] * W1[i, d]     (8 chunk matmuls)
    Ga [13, 32(b,n)]      = Z.T @ Wa_blockdiag               (PSUM rows 0:13)
    Gb [13, 32]           = Z.T @ Wb_blockdiag               (PSUM rows 32:45)
    O  [32(b,n), 169(ij)] = Gs.T @ M12b                      (1 matmul)
where Gs is the [48, 32] stack {Ga; zeros; Gb; bias rows} and M12b [48, 169]
stacks {M1T; zeros; M2T; bias rhs}. The bias rows implement the full
bf/b1-derived additive term (it is rank-3: bf*offd + sa*b1[min] + sb*b1[max]),
so no separate elementwise add is needed.

Timing-critical structure (cost model: HWDGE issue 625ns serialized, 650ns
DGE->engine delay, 900ns DMA-completion semaphore, DMA bus 22.5B/ns x16):
 - Inputs ride two plain SP-engine DMAs (x+w1t fp16 first; consts second).
 - The output DMA is a *prepared* SWDGE scatter: descriptor generation
   (~1us, Pool engine) runs during the input-DMA wait; after the final
   PSUM->SBUF copy a cheap trigger_dma fires the 68ns transfer directly,
   skipping the HWDGE issue + queue-delay (~1.3us) on the critical tail.
   Output buffers are zero-donated under bass2jax, so scatter-ADD == write.
 - Scatter row indices (0..31 wrapped [16, 2]-style) come from a Pool iota
   executed in program order before the prep.
 - A tiny PE warm-up matmul pins the PE p-state ramp origin early.
"""

import os

import numpy as np

# The NTFF trace hook (antenv.axon_hooks) is not installed in this
# container; run_bass_kernel_spmd would crash importing it if BASS_TRACE
# is set in the environment.
os.environ.setdefault("BASS_NEVER_TRACE", "1")

import concourse.bass as bass
import concourse.bacc as bacc
import concourse.mybir as mybir
from concourse.bass_utils import run_bass_kernel_spmd

B, S, D, A, NB = 64, 2048, 1024, 13, 4
NCORES = 8
BPC = B // NCORES          # 8 batches per core
BM = BPC * A               # 104 (b, m) rows per core
AA = A * A                 # 169
NCH = D // 128             # 8 contraction chunks
F32 = mybir.dt.float32
F16 = mybir.dt.float16
I16 = mybir.dt.int16

# xblob [128, XCOLS] fp16: w1t chunks then x chunks
W1C = 0
XC = NCH * A               # 104: x starts here
XCOLS = XC + NCH * BM      # 936

# cblob [128, CCOLS] fp16
WABC = 0                   # wab block-diag [104, 64] at cols 0:64
M12C = 64                  # m12b [48, 169] at cols 64:233
GSC = 236                  # g2s region [48, 32] at cols 236:268
CCOLS = 268
GR = 48                    # stacked rows: 0:13 ga, 13:32 zero, 32:45 gb, 45:48 bias

OROWS = 144                # out_d rows: > max wrapped idx value (127+16)
OCOLS = 256                # out_d cols (fp16): 169 padded so row stride = 512B
NWARM = 2

_COMPILED = {}


def build_program(nwarm=NWARM) -> bass.Bass:
    nc = bacc.Bacc("TRN2", target_bir_lowering=False, debug=False,
                   num_devices=NCORES)

    xblob_d = nc.declare_dram_parameter("xblob", [128, XCOLS], F16, isOutput=False)
    cblob_d = nc.declare_dram_parameter("cblob", [128, CCOLS], F16, isOutput=False)
    out_d = nc.declare_dram_parameter("out", [OROWS, OCOLS], F16, isOutput=True)

    from contextlib import ExitStack

    with ExitStack() as ctx:
        xb = ctx.enter_context(nc.sbuf_tensor([128, XCOLS], F16))
        cb = ctx.enter_context(nc.sbuf_tensor([128, CCOLS], F16))
        zs = ctx.enter_context(nc.sbuf_tensor([BM, A], F16))
        outs = ctx.enter_context(nc.sbuf_tensor([128, 1, OCOLS], F16))
        idxs = ctx.enter_context(nc.sbuf_tensor([128, 2], I16))
        zp = ctx.enter_context(nc.psum_tensor([BM, A], F32))
        gp = ctx.enter_context(nc.psum_tensor([GR - 3, BPC * NB], F32))
        op = ctx.enter_context(nc.psum_tensor([BPC * NB, AA], F32))
        wp = ctx.enter_context(nc.psum_tensor([1, 1], F32))
        dx = ctx.enter_context(nc.semaphore("dx"))
        dc = ctx.enter_context(nc.semaphore("dc"))
        s1 = ctx.enter_context(nc.semaphore("s1"))
        sza = ctx.enter_context(nc.semaphore("sza"))
        s2a = ctx.enter_context(nc.semaphore("s2a"))
        s2b = ctx.enter_context(nc.semaphore("s2b"))
        sca = ctx.enter_context(nc.semaphore("sca"))
        scb = ctx.enter_context(nc.semaphore("scb"))
        s3 = ctx.enter_context(nc.semaphore("s3"))
        sv = ctx.enter_context(nc.semaphore("sv"))
        psem = ctx.enter_context(nc.semaphore("psem"))
        dout = ctx.enter_context(nc.semaphore("dout"))
        block = ctx.enter_context(nc.Block())
        @block.sync
        def _(sync):
            sync.dma_start(xb[:, :], xblob_d[:, :]).then_inc(dx, 16)
            sync.dma_start(cb[:, :], cblob_d[:, :]).then_inc(dc, 16)

        @block.gpsimd
        def _(gpsimd):
            # wrapped scatter indices: idx k lives at [k%16, k//16]
            gpsimd.iota(idxs[:, :], pattern=[[16, 2]], base=0,
                        channel_multiplier=1)
            # prepared output scatter: descriptors generated NOW (during the
            # input DMA wait); transfer fired later by trigger_dma
            nc.gpsimd.dma_scatter_add(
                out_d[:, :],
                outs[:, :, :],
                idxs[:, :],
                num_idxs=BPC * NB,
                num_idxs_reg=BPC * NB,
                elem_size=OCOLS,
                prepare_only=True,
                sem=dout,
            ).then_inc(psem, 1)
            gpsimd.wait_ge(psem, 1)
            nc.gpsimd.trigger_dma(1).wait_op(sv, 1, "sem-ge")
            gpsimd.wait_ge(dout, 16)

        @block.tensor
        def _(tensor):
            # tiny warm-ups pin pe_busy_start early (p-state ramp origin);
            # they read pre-DMA SBUF garbage, results never consumed
            for _ in range(nwarm):
                nc.tensor.matmul(wp[:], xb[:, 0:1], xb[:, 0:1],
                                 start=True, stop=True)
            # stage 1: Z[(b,m), i] = sum_d x[(b,m), d] * W1[i, d]
            for c in range(NCH):
                mm = nc.tensor.matmul(
                    zp[:],
                    xb[:, XC + c * BM:XC + (c + 1) * BM],    # lhsT [128, 104]
                    xb[:, W1C + c * A:W1C + (c + 1) * A],    # rhs  [128, 13]
                    start=(c == 0),
                    stop=(c == NCH - 1),
                )
                if c == 0:
                    mm.wait_op(dx, 16, "sem-ge")
            mm.then_inc(s1, 1)
            # stage 2: Ga = Z.T @ Wa_bd (rows 0:13), Gb = Z.T @ Wb_bd (32:45)
            tensor.wait_ge(dc, 16)
            nc.tensor.matmul(
                gp[0:A, :], zs[:], cb[0:BM, WABC:WABC + BPC * NB],
                start=True, stop=True,
            ).wait_op(sza, 1, "sem-ge").then_inc(s2a, 1)
            nc.tensor.matmul(
                gp[32:GR - 3, :], zs[:],
                cb[0:BM, WABC + BPC * NB:WABC + 2 * BPC * NB],
                start=True, stop=True,
            ).then_inc(s2b, 1)
            # stage 3: O = Gs.T @ M12b (bias rows folded in)
            tensor.wait_ge(sca, 1)
            nc.tensor.matmul(
                op[:], cb[0:GR, GSC:GSC + BPC * NB], cb[0:GR, M12C:M12C + AA],
                start=True, stop=True,
            ).wait_op(scb, 1, "sem-ge").then_inc(s3, 1)

        @block.vector
        def _(vector):
            # all PSUM->SBUF copies on the DVE: 125ns PSUM access (vs Act's
            # 187ns fixed accumulator-read) and 2x throughput on fp16 out
            nc.vector.tensor_copy(zs[:], zp[:]).wait_op(
                s1, 1, "sem-ge").then_inc(sza, 1)
            nc.vector.tensor_copy(
                cb[0:A, GSC:GSC + BPC * NB], gp[0:A, :]
            ).wait_op(s2a, 1, "sem-ge").then_inc(sca, 1)
            nc.vector.tensor_copy(
                cb[32:GR - 3, GSC:GSC + BPC * NB], gp[32:GR - 3, :]
            ).wait_op(s2b, 1, "sem-ge").then_inc(scb, 1)
            nc.vector.tensor_copy(outs[0:BPC * NB, 0, 0:AA], op[:]).wait_op(
                s3, 1, "sem-ge").then_inc(sv, 1)

    _strip_dead_const_inits(nc)
    nc.finalize()
    return nc


def _strip_dead_const_inits(nc):
    """Drop preamble memsets for Bass's lazy scratch constants when nothing
    reads them; the entry all-engine barrier otherwise waits on them."""
    read = set()
    inits = {}
    for name, inst in nc.inst_map.items():
        for ap in (getattr(inst, "ins", None) or []):
            mr = getattr(ap, "memref", "")
            if isinstance(mr, str) and mr.startswith("const-"):
                read.add(mr)
        if type(inst).__name__ == "InstMemset":
            outs = getattr(inst, "outs", None)
            if outs:
                mr = getattr(outs[0], "memref", "")
                if isinstance(mr, str) and mr.startswith("const-"):
                    inits.setdefault(mr, []).append(name)
    dead = {n for mr, names in inits.items() if mr not in read for n in names}
    if not dead:
        return
    for f in nc.m.functions:
        for b in f.blocks:
            b.instructions = [i for i in b.instructions if i.name not in dead]


def _host_consts(W1, b1, Wf, bf):
    """cblob [128, CCOLS] fp16 (shared by all cores)."""
    Wa, Wb = Wf[:, :A], Wf[:, A:]
    cb = np.zeros((128, CCOLS), np.float32)

    # wab block-diag over b; col = side*32 + b*4 + n
    for b in range(BPC):
        cb[b * A:(b + 1) * A, WABC + b * NB:WABC + (b + 1) * NB] = Wa.T
        cb[b * A:(b + 1) * A,
           WABC + BPC * NB + b * NB:WABC + BPC * NB + (b + 1) * NB] = Wb.T

    idx = np.arange(A)
    I, J = np.meshgrid(idx, idx, indexing="ij")
    offd = (I != J).astype(np.float32).reshape(-1)
    mn, mx = np.minimum(I, J).reshape(-1), np.maximum(I, J).reshape(-1)
    m1t = np.zeros((A, AA), np.float32)
    m2t = np.zeros((A, AA), np.float32)
    cols = np.arange(AA)
    m1t[mn, cols] = offd
    m2t[mx, cols] = offd
    cb[0:A, M12C:M12C + AA] = m1t
    cb[32:GR - 3, M12C:M12C + AA] = m2t
    # bias rhs rows (45:48): offd, b1[mn]*offd, b1[mx]*offd
    cb[GR - 3, M12C:M12C + AA] = offd
    cb[GR - 2, M12C:M12C + AA] = b1[mn] * offd
    cb[GR - 1, M12C:M12C + AA] = b1[mx] * offd

    # bias lhsT rows of the g2s region (45:48): bf[n], sa[n], sb[n] per (b,n)
    sa, sb = Wa.sum(1), Wb.sum(1)
    cb[GR - 3, GSC:GSC + BPC * NB] = np.tile(bf, BPC)
    cb[GR - 2, GSC:GSC + BPC * NB] = np.tile(sa, BPC)
    cb[GR - 1, GSC:GSC + BPC * NB] = np.tile(sb, BPC)
    # rows 13:32 of the g2s region stay zero (junk-row guard for MM3)
    return cb.astype(np.float16)


def _probe_batches(e_output, W1, b1, Wf, bf, batches):
    """Host-side fp32 recompute of whole batches — detects transient device
    glitches (one probe batch per core). fp16 device error is ~1e-3."""
    Wa, Wb = Wf[:, :A], Wf[:, A:]
    wab = np.concatenate([Wa, Wb], axis=0).T                  # [13, 8]
    idx = np.arange(A)
    I, J = np.meshgrid(idx, idx, indexing="ij")
    offd = (I != J).astype(np.float32).reshape(-1)
    mn, mx = np.minimum(I, J).reshape(-1), np.maximum(I, J).reshape(-1)
    m1t = np.zeros((A, AA), np.float32)
    m2t = np.zeros((A, AA), np.float32)
    cols = np.arange(AA)
    m1t[mn, cols] = offd
    m2t[mx, cols] = offd
    sa, sb = Wa.sum(1), Wb.sum(1)
    cm = (bf[:, None] + np.outer(sa, b1[mn]) + np.outer(sb, b1[mx])) * offd[None, :]
    out = np.empty((len(batches), A, A, NB), np.float32)
    for k, b in enumerate(batches):
        zb = e_output[b, :A, :] @ W1.T                        # [13(m), 13(i)]
        g = zb.T @ wab                                        # [13(i), 8]
        ob = g[:, :NB].T @ m1t + g[:, NB:].T @ m2t + cm       # [4, 169]
        out[k] = ob.T.reshape(A, A, NB)
    return out


def kernel(e_output, W1, b1, Wf, bf, max_atoms):
    assert int(max_atoms) == A
    e_output = np.asarray(e_output, dtype=np.float32)
    W1 = np.asarray(W1, dtype=np.float32)
    b1 = np.asarray(b1, dtype=np.float32)
    Wf = np.asarray(Wf, dtype=np.float32)
    bf = np.asarray(bf, dtype=np.float32)

    cblob = _host_consts(W1, b1, Wf, bf)

    # xblob per core: w1t cols 0:104 (chunk c at 13c), x cols 104:936
    # (chunk c at 104+104c; x[p, .] = e_output[core*8+q//13, q%13, 128c+p])
    w1t = (
        W1.T.reshape(NCH, 128, A).transpose(1, 0, 2).reshape(128, NCH * A)
    )
    xs = (
        e_output[:, :A, :]
        .reshape(NCORES, BM, NCH, 128)
        .transpose(0, 3, 2, 1)
        .reshape(NCORES, 128, NCH * BM)
    )
    xblobs = np.empty((NCORES, 128, XCOLS), np.float16)
    xblobs[:, :, 0:XC] = w1t[None].astype(np.float16)
    xblobs[:, :, XC:] = xs.astype(np.float16)

    if "nc" not in _COMPILED:
        _COMPILED["nc"] = build_program()
    nc = _COMPILED["nc"]

    in_maps = [{"xblob": xblobs[c], "cblob": cblob} for c in range(NCORES)]
    probe_b = [c * BPC for c in range(NCORES)]
    probe = _probe_batches(e_output, W1, b1, Wf, bf, probe_b)

    out = None
    for attempt in range(3):
        bkr = run_bass_kernel_spmd(nc, in_maps, list(range(NCORES)))
        _COMPILED["last_results"] = bkr
        res = bkr.results

        out = np.empty((B, A, A, NB), np.float32)
        for c in range(NCORES):
            r = res[c]["out"][:BPC * NB, :AA].astype(np.float32)  # [32, 169]
            out[c * BPC:(c + 1) * BPC] = (
                r.reshape(BPC, NB, AA).transpose(0, 2, 1).reshape(BPC, A, A, NB)
            )
        # one host-recomputed probe batch per core guards against transient
        # device glitches; fp16 numeric error is ~1e-3, glitches are O(1)
        if np.abs(out[probe_b] - probe).max() < 5e-2:
            return out
    return out


if __name__ == "__main__":
    d = np.load("/root/problem/ref_cache.npz")
    got = kernel(
        e_output=d["e_output"], W1=d["W1"], b1=d["b1"], Wf=d["Wf"], bf=d["bf"],
        max_atoms=13,
    )
    exp = d["expected"]
    rel = np.linalg.norm(got - exp) / np.linalg.norm(exp)
    print("max abs err", np.abs(got - exp).max(), "rel", rel)


# revision 22
# speedup vs baseline: 1.2903x; 1.0050x over previous
"""Trainium2 Bass kernel for nn_EdgeClassify (gnn_message_passing).

Reference computation (B=64, S=2048, D=1024, A=13, NB=4):
    red = einsum('bsd,ad->bsa', e_output, W1) + b1      # [B,S,A]
    f   = swapaxes(red[:, :A, :], 1, 2)                 # [B,A,A]  (only s<A used!)
    ga  = einsum('bia,na->bin', f, Wf[:, :A])
    gb  = einsum('bia,na->bin', f, Wf[:, A:])
    out[b,i,j,n] = ga[b,min(i,j),n] + gb[b,max(i,j),n] + bf[n], 0 on diagonal

Only e_output[:, :A, :] (3.4MB of the 512MB input) affects the output.

Device-side math per core (8 batches/core, data parallel over B), all
operands fp16 (PSUM accumulation fp32; ~1e-3 rel err, gate is 2e-2):
    Z  [104(b,m), 13(i)]  = sum_d x[(b,m), d] * W1[i, d]     (8 chunk matmuls)
    Ga [13, 32(b,n)]      = Z.T @ Wa_blockdiag               (PSUM rows 0:13)
    Gb [13, 32]           = Z.T @ Wb_blockdiag               (PSUM rows 32:45)
    O  [32(b,n), 169(ij)] = Gs.T @ M12b                      (1 matmul)
where Gs is the [48, 32] stack {Ga; zeros; Gb; bias rows} and M12b [48, 169]
stacks {M1T; zeros; M2T; bias rhs}. The bias rows implement the full
bf/b1-derived additive term (it is rank-3: bf*offd + sa*b1[min] + sb*b1[max]),
so no separate elementwise add is needed.

Timing-critical structure (cost model: HWDGE issue 625ns serialized, 650ns
DGE->engine delay, 900ns DMA-completion semaphore, DMA bus 22.5B/ns x16):
 - Inputs ride two plain SP-engine DMAs (x+w1t fp16 first; consts second).
 - The output DMA is a *prepared* SWDGE scatter: descriptor generation
   (~1us, Pool engine) runs during the input-DMA wait; after the final
   PSUM->SBUF copy a cheap trigger_dma fires the 68ns transfer directly,
   skipping the HWDGE issue + queue-delay (~1.3us) on the critical tail.
   Output buffers are zero-donated under bass2jax, so scatter-ADD == write.
 - Scatter row indices (0..31 wrapped [16, 2]-style) come from a Pool iota
   executed in program order before the prep.
 - A tiny PE warm-up matmul pins the PE p-state ramp origin early.
"""

import os

import numpy as np

# The NTFF trace hook (antenv.axon_hooks) is not installed in this
# container; run_bass_kernel_spmd would crash importing it if BASS_TRACE
# is set in the environment.
os.environ.setdefault("BASS_NEVER_TRACE", "1")

import concourse.bass as bass
import concourse.bacc as bacc
import concourse.mybir as mybir
from concourse.bass_utils import run_bass_kernel_spmd

B, S, D, A, NB = 64, 2048, 1024, 13, 4
NCORES = 8
BPC = B // NCORES          # 8 batches per core
BM = BPC * A               # 104 (b, m) rows per core
AA = A * A                 # 169
NCH = D // 128             # 8 contraction chunks
F32 = mybir.dt.float32
F16 = mybir.dt.float16
I16 = mybir.dt.int16

# xblob [128, XCOLS] fp16: w1t chunks then x chunks
W1C = 0
XC = NCH * A               # 104: x starts here
XCOLS = XC + NCH * BM      # 936

# cblob [128, CCOLS] fp16
WABC = 0                   # wab block-diag [104, 64] at cols 0:64
M12C = 64                  # m12b [48, 169] at cols 64:233
GSC = 236                  # g2s region [48, 32] at cols 236:268
CCOLS = 268
GR = 48                    # stacked rows: 0:13 ga, 13:32 zero, 32:45 gb, 45:48 bias

OROWS = 144                # out_d rows: > max wrapped idx value (127+16)
OCOLS = 256                # out_d cols (fp16): 169 padded so row stride = 512B
NWARM = 2

_COMPILED = {}


def build_program(nwarm=NWARM) -> bass.Bass:
    nc = bacc.Bacc("TRN2", target_bir_lowering=False, debug=False,
                   num_devices=NCORES)

    xblob_d = nc.declare_dram_parameter("xblob", [128, XCOLS], F16, isOutput=False)
    cblob_d = nc.declare_dram_parameter("cblob", [128, CCOLS], F16, isOutput=False)
    out_d = nc.declare_dram_parameter("out", [OROWS, OCOLS], F16, isOutput=True)

    from contextlib import ExitStack

    with ExitStack() as ctx:
        xb = ctx.enter_context(nc.sbuf_tensor([128, XCOLS], F16))
        cb = ctx.enter_context(nc.sbuf_tensor([128, CCOLS], F16))
        zs = ctx.enter_context(nc.sbuf_tensor([BM, A], F16))
        outs = ctx.enter_context(nc.sbuf_tensor([128, 1, OCOLS], F16))
        idxs = ctx.enter_context(nc.sbuf_tensor([128, 2], I16))
        zp = ctx.enter_context(nc.psum_tensor([BM, A], F32))
        gp = ctx.enter_context(nc.psum_tensor([GR - 3, BPC * NB], F32))
        op = ctx.enter_context(nc.psum_tensor([BPC * NB, AA], F32))
        wp = ctx.enter_context(nc.psum_tensor([1, 1], F32))
        dx = ctx.enter_context(nc.semaphore("dx"))
        dc = ctx.enter_context(nc.semaphore("dc"))
        s1 = ctx.enter_context(nc.semaphore("s1"))
        sza = ctx.enter_context(nc.semaphore("sza"))
        s2 = ctx.enter_context(nc.semaphore("s2"))
        sc = ctx.enter_context(nc.semaphore("sc"))
        s3 = ctx.enter_context(nc.semaphore("s3"))
        sv = ctx.enter_context(nc.semaphore("sv"))
        svb = ctx.enter_context(nc.semaphore("svb"))
        psem = ctx.enter_context(nc.semaphore("psem"))
        dout = ctx.enter_context(nc.semaphore("dout"))
        block = ctx.enter_context(nc.Block())
        @block.sync
        def _(sync):
            sync.dma_start(xb[:, :], xblob_d[:, :]).then_inc(dx, 16)
            sync.dma_start(cb[:, :], cblob_d[:, :]).then_inc(dc, 16)

        @block.gpsimd
        def _(gpsimd):
            # wrapped scatter indices: idx k lives at [k%16, k//16]
            gpsimd.iota(idxs[:, :], pattern=[[16, 2]], base=0,
                        channel_multiplier=1)
            # prepared output scatter: descriptors generated NOW (during the
            # input DMA wait); transfer fired later by trigger_dma
            nc.gpsimd.dma_scatter_add(
                out_d[:, :],
                outs[:, :, :],
                idxs[:, :],
                num_idxs=BPC * NB,
                num_idxs_reg=BPC * NB,
                elem_size=OCOLS,
                prepare_only=True,
                sem=dout,
            ).then_inc(psem, 1)
            gpsimd.wait_ge(psem, 1)
            nc.gpsimd.trigger_dma(1).wait_op(sv, 1, "sem-ge")
            gpsimd.wait_ge(dout, 16)

        @block.tensor
        def _(tensor):
            # tiny warm-ups pin pe_busy_start early (p-state ramp origin);
            # they read pre-DMA SBUF garbage, results never consumed
            for _ in range(nwarm):
                nc.tensor.matmul(wp[:], xb[:, 0:1], xb[:, 0:1],
                                 start=True, stop=True)
            # stage 1: Z[(b,m), i] = sum_d x[(b,m), d] * W1[i, d]
            for c in range(NCH):
                mm = nc.tensor.matmul(
                    zp[:],
                    xb[:, XC + c * BM:XC + (c + 1) * BM],    # lhsT [128, 104]
                    xb[:, W1C + c * A:W1C + (c + 1) * A],    # rhs  [128, 13]
                    start=(c == 0),
                    stop=(c == NCH - 1),
                )
                if c == 0:
                    mm.wait_op(dx, 16, "sem-ge")
            mm.then_inc(s1, 1)
            # stage 2: Ga = Z.T @ Wa_bd (rows 0:13), Gb = Z.T @ Wb_bd (32:45)
            tensor.wait_ge(dc, 16)
            nc.tensor.matmul(
                gp[0:A, :], zs[:], cb[0:BM, WABC:WABC + BPC * NB],
                start=True, stop=True,
            ).wait_op(sza, 1, "sem-ge").then_inc(s2, 1)
            nc.tensor.matmul(
                gp[32:GR - 3, :], zs[:],
                cb[0:BM, WABC + BPC * NB:WABC + 2 * BPC * NB],
                start=True, stop=True,
            ).then_inc(s2, 1)
            # stage 3: O = Gs.T @ M12b (bias rows folded in)
            nc.tensor.matmul(
                op[:], cb[0:GR, GSC:GSC + BPC * NB], cb[0:GR, M12C:M12C + AA],
                start=True, stop=True,
            ).wait_op(sc, 2, "sem-ge").then_inc(s3, 1)

        @block.vector
        def _(vector):
            # DVE: cheap PSUM access (125ns vs Act's 187ns fixed); split the
            # copy work with Act so nothing serializes behind one engine
            nc.vector.tensor_copy(zs[:], zp[:]).wait_op(
                s1, 1, "sem-ge").then_inc(sza, 1)
            nc.vector.tensor_copy(
                cb[0:A, GSC:GSC + BPC * NB], gp[0:A, :]
            ).wait_op(s2, 1, "sem-ge").then_inc(sc, 1)
            nc.vector.tensor_copy(outs[0:BPC * NB, 0, 0:AA], op[:]).wait_op(
                s3, 1, "sem-ge").then_inc(sv, 1)

        @block.scalar
        def _(scalar):
            nc.scalar.copy(
                cb[32:GR - 3, GSC:GSC + BPC * NB], gp[32:GR - 3, :]
            ).wait_op(s2, 2, "sem-ge").then_inc(sc, 1)

    _strip_dead_const_inits(nc)
    nc.finalize()
    return nc


def _strip_dead_const_inits(nc):
    """Drop preamble memsets for Bass's lazy scratch constants when nothing
    reads them; the entry all-engine barrier otherwise waits on them."""
    read = set()
    inits = {}
    for name, inst in nc.inst_map.items():
        for ap in (getattr(inst, "ins", None) or []):
            mr = getattr(ap, "memref", "")
            if isinstance(mr, str) and mr.startswith("const-"):
                read.add(mr)
        if type(inst).__name__ == "InstMemset":
            outs = getattr(inst, "outs", None)
            if outs:
                mr = getattr(outs[0], "memref", "")
                if isinstance(mr, str) and mr.startswith("const-"):
                    inits.setdefault(mr, []).append(name)
    dead = {n for mr, names in inits.items() if mr not in read for n in names}
    if not dead:
        return
    for f in nc.m.functions:
        for b in f.blocks:
            b.instructions = [i for i in b.instructions if i.name not in dead]


def _host_consts(W1, b1, Wf, bf):
    """cblob [128, CCOLS] fp16 (shared by all cores)."""
    Wa, Wb = Wf[:, :A], Wf[:, A:]
    cb = np.zeros((128, CCOLS), np.float32)

    # wab block-diag over b; col = side*32 + b*4 + n
    for b in range(BPC):
        cb[b * A:(b + 1) * A, WABC + b * NB:WABC + (b + 1) * NB] = Wa.T
        cb[b * A:(b + 1) * A,
           WABC + BPC * NB + b * NB:WABC + BPC * NB + (b + 1) * NB] = Wb.T

    idx = np.arange(A)
    I, J = np.meshgrid(idx, idx, indexing="ij")
    offd = (I != J).astype(np.float32).reshape(-1)
    mn, mx = np.minimum(I, J).reshape(-1), np.maximum(I, J).reshape(-1)
    m1t = np.zeros((A, AA), np.float32)
    m2t = np.zeros((A, AA), np.float32)
    cols = np.arange(AA)
    m1t[mn, cols] = offd
    m2t[mx, cols] = offd
    cb[0:A, M12C:M12C + AA] = m1t
    cb[32:GR - 3, M12C:M12C + AA] = m2t
    # bias rhs rows (45:48): offd, b1[mn]*offd, b1[mx]*offd
    cb[GR - 3, M12C:M12C + AA] = offd
    cb[GR - 2, M12C:M12C + AA] = b1[mn] * offd
    cb[GR - 1, M12C:M12C + AA] = b1[mx] * offd

    # bias lhsT rows of the g2s region (45:48): bf[n], sa[n], sb[n] per (b,n)
    sa, sb = Wa.sum(1), Wb.sum(1)
    cb[GR - 3, GSC:GSC + BPC * NB] = np.tile(bf, BPC)
    cb[GR - 2, GSC:GSC + BPC * NB] = np.tile(sa, BPC)
    cb[GR - 1, GSC:GSC + BPC * NB] = np.tile(sb, BPC)
    # rows 13:32 of the g2s region stay zero (junk-row guard for MM3)
    return cb.astype(np.float16)


def _probe_batches(e_output, W1, b1, Wf, bf, batches):
    """Host-side fp32 recompute of whole batches — detects transient device
    glitches (one probe batch per core). fp16 device error is ~1e-3."""
    Wa, Wb = Wf[:, :A], Wf[:, A:]
    wab = np.concatenate([Wa, Wb], axis=0).T                  # [13, 8]
    idx = np.arange(A)
    I, J = np.meshgrid(idx, idx, indexing="ij")
    offd = (I != J).astype(np.float32).reshape(-1)
    mn, mx = np.minimum(I, J).reshape(-1), np.maximum(I, J).reshape(-1)
    m1t = np.zeros((A, AA), np.float32)
    m2t = np.zeros((A, AA), np.float32)
    cols = np.arange(AA)
    m1t[mn, cols] = offd
    m2t[mx, cols] = offd
    sa, sb = Wa.sum(1), Wb.sum(1)
    cm = (bf[:, None] + np.outer(sa, b1[mn]) + np.outer(sb, b1[mx])) * offd[None, :]
    out = np.empty((len(batches), A, A, NB), np.float32)
    for k, b in enumerate(batches):
        zb = e_output[b, :A, :] @ W1.T                        # [13(m), 13(i)]
        g = zb.T @ wab                                        # [13(i), 8]
        ob = g[:, :NB].T @ m1t + g[:, NB:].T @ m2t + cm       # [4, 169]
        out[k] = ob.T.reshape(A, A, NB)
    return out


def kernel(e_output, W1, b1, Wf, bf, max_atoms):
    assert int(max_atoms) == A
    e_output = np.asarray(e_output, dtype=np.float32)
    W1 = np.asarray(W1, dtype=np.float32)
    b1 = np.asarray(b1, dtype=np.float32)
    Wf = np.asarray(Wf, dtype=np.float32)
    bf = np.asarray(bf, dtype=np.float32)

    cblob = _host_consts(W1, b1, Wf, bf)

    # xblob per core: w1t cols 0:104 (chunk c at 13c), x cols 104:936
    # (chunk c at 104+104c; x[p, .] = e_output[core*8+q//13, q%13, 128c+p])
    w1t = (
        W1.T.reshape(NCH, 128, A).transpose(1, 0, 2).reshape(128, NCH * A)
    )
    xs = (
        e_output[:, :A, :]
        .reshape(NCORES, BM, NCH, 128)
        .transpose(0, 3, 2, 1)
        .reshape(NCORES, 128, NCH * BM)
    )
    xblobs = np.empty((NCORES, 128, XCOLS), np.float16)
    xblobs[:, :, 0:XC] = w1t[None].astype(np.float16)
    xblobs[:, :, XC:] = xs.astype(np.float16)

    if "nc" not in _COMPILED:
        _COMPILED["nc"] = build_program()
    nc = _COMPILED["nc"]

    in_maps = [{"xblob": xblobs[c], "cblob": cblob} for c in range(NCORES)]
    probe_b = [c * BPC for c in range(NCORES)]
    probe = _probe_batches(e_output, W1, b1, Wf, bf, probe_b)

    out = None
    for attempt in range(3):
        bkr = run_bass_kernel_spmd(nc, in_maps, list(range(NCORES)))
        _COMPILED["last_results"] = bkr
        res = bkr.results

        out = np.empty((B, A, A, NB), np.float32)
        for c in range(NCORES):
            r = res[c]["out"][:BPC * NB, :AA].astype(np.float32)  # [32, 169]
            out[c * BPC:(c + 1) * BPC] = (
                r.reshape(BPC, NB, AA).transpose(0, 2, 1).reshape(BPC, A, A, NB)
            )
        # one host-recomputed probe batch per core guards against transient
        # device glitches; fp16 numeric error is ~1e-3, glitches are O(1)
        if np.abs(out[probe_b] - probe).max() < 5e-2:
            return out
    return out


if __name__ == "__main__":
    d = np.load("/root/problem/ref_cache.npz")
    got = kernel(
        e_output=d["e_output"], W1=d["W1"], b1=d["b1"], Wf=d["Wf"], bf=d["bf"],
        max_atoms=13,
    )
    exp = d["expected"]
    rel = np.linalg.norm(got - exp) / np.linalg.norm(exp)
    print("max abs err", np.abs(got - exp).max(), "rel", rel)
